# revision 53
# baseline (speedup 1.0000x reference)
"""Trainium2 Bass kernel for a Neural ODE (dopri5, fixed substeps) — v3.

Problem: B=1024 trajectories of a D=64-dim ODE driven by an MLP
f(t,x) = tanh([x,u(t)] @ W1) @ W2, integrated with Dormand-Prince RK45
over 49 intervals x 4 substeps = 196 steps (6 MLP evals each).

Strategy (pure batch data-parallel, 8 cores x 128 batch), fully unrolled:
- Transposed layout: state xT [64,128] f32 (batch on free dim), stage
  inputs zT [72,128] bf16 (64 state rows + 8 forcing rows), hidden
  hT [128,2,128] bf16.
- Single-bf16 weights (no hi/lo split): tolerance is 2e-2, measured
  numerics land at ~1.4e-3.
- Forcing is interpolated ON DEVICE: raw u (transposed [8,TU,128] bf16)
  is preloaded to SBUF, diffs are built with one shifted tensor-sub;
  each stage's u rows are two Pool ops with the interpolation weight
  baked as an immediate (t grids are known at build time; the program
  is cached on them).
- The last TWO RK terms of each stage's z are fused into its h_pre
  PSUM group via c*M matmuls (M = W2 @ W1x, built on device in the
  prologue): the m2 set (second-to-last term) starts each group two
  stages early, the m1 set closes it, keeping the serial chain at
  tanh -> 4 matmuls and giving every z x-row write (and the group's
  W1 openers) a full stage of slack against the in-order PE and the
  DVE queue. Stage periods sit at the 757 ns chain floor.
- Fully unrolled 196-step program: no For_i (no all-engine barriers, no
  per-iteration act-table reloads), no dynamic DMA descriptors. Outputs
  accumulate in SBUF (f16) and leave in ONE big DMA at the end; total
  DMA count is 6.
- RK linear combinations are DVE scalar_tensor_tensor AXPYs reading f
  straight from PSUM, with dt-scaled tableau coefficients baked as
  immediates.
"""

import os
import numpy as np
import ml_dtypes

import concourse.bass as bass
import concourse.bacc as bacc
import concourse.mybir as mybir
import concourse.tile as tile
from concourse.bass_utils import run_bass_kernel_spmd
from concourse.bass_interp import get_hw_module

NCORES = 8
B, D, F, H = 1024, 64, 8, 256
T, TU, N_SUB = 50, 128, 4
NITER = int(os.environ.get('NODE_NITER', T - 1))  # build fewer iters (dev)
BC = B // NCORES                   # 128 batch per core
KZ = D + F                         # 72 = state + forcing rows
HH = H // 2                        # 128

f32 = mybir.dt.float32
f16 = mybir.dt.float16
bf16 = mybir.dt.bfloat16
SUB = mybir.AluOpType.subtract
FP = mybir.ActivationFunctionType
MULT = mybir.AluOpType.mult
ADD = mybir.AluOpType.add

A_TAB = [
    [],
    [1 / 5],
    [3 / 40, 9 / 40],
    [44 / 45, -56 / 15, 32 / 9],
    [19372 / 6561, -25360 / 2187, 64448 / 6561, -212 / 729],
    [9017 / 3168, -355 / 33, 46732 / 5247, 49 / 176, -5103 / 18656],
]
B_TAB = [35 / 384, 0.0, 500 / 1113, 125 / 192, -2187 / 6784, 11 / 84]

_CACHE = {}
LAST_RESULTS = None


def _host_times(t_eval):
    """Substep times/dts exactly as the fp32 reference computes them."""
    t_eval = np.asarray(t_eval, np.float32)
    dtc = np.diff(t_eval)
    frac = (np.arange(N_SUB, dtype=np.float32) / np.float32(N_SUB))
    ts = (t_eval[:-1, None] + dtc[:, None] * frac).reshape(-1)
    dts = np.repeat(dtc / np.float32(N_SUB), N_SUB)
    return ts.astype(np.float32), dts.astype(np.float32)


def _stage_times(t, dt):
    """The 6 stage eval times for one step, mirroring the reference fp32."""
    t = np.float32(t)
    dt = np.float32(dt)
    return [
        t,
        t + dt / np.float32(5),
        t + np.float32(3) * dt / np.float32(10),
        t + np.float32(4) * dt / np.float32(5),
        t + np.float32(8) * dt / np.float32(9),
        t + dt,
    ]


def _interp_consts(t_eval, t_u):
    """(idx, w) fp32 interpolation constants for all NITER*N_SUB*6 stages."""
    ts, dts = _host_times(t_eval)
    nstage = NITER * N_SUB * 6
    tq = np.empty(nstage, np.float32)
    for s in range(NITER * N_SUB):
        tq[s * 6:(s + 1) * 6] = _stage_times(ts[s], dts[s])
    idx = np.clip(np.searchsorted(t_u, tq, side="right") - 1, 0, TU - 2)
    w = ((tq - t_u[idx]) / (t_u[idx + 1] - t_u[idx])).astype(np.float32)
    return idx.astype(np.int64), w


def _build_program(dt, idx, w):
    """Build the SPMD Bass program (identical on all cores), fully unrolled.

    dt: constant substep size baked into RK coefficients.
    idx/w: per-stage forcing interpolation constants (baked).
    """
    nc = bacc.Bacc("TRN2", target_bir_lowering=False, debug=False,
                   enable_asserts=False)

    x0T_d = nc.dram_tensor("x0T", [D, BC], f32, kind="ExternalInput")
    uT_d = nc.dram_tensor("uT", [F, TU, BC], bf16, kind="ExternalInput")
    duT_d = nc.dram_tensor("duT", [F, TU, BC], bf16, kind="ExternalInput")
    w1_d = nc.dram_tensor("w1", [KZ, H], bf16, kind="ExternalInput")
    w2k_d = nc.dram_tensor("w2k", [HH, 2, D], bf16, kind="ExternalInput")
    w2t_d = nc.dram_tensor("w2t", [D, H], bf16, kind="ExternalInput")
    # f16 output: halves device<->host traffic; the f32 state chain stays
    # on-device, outputs are rounded snapshots only
    out_d = nc.dram_tensor("outT", [D, NITER, BC], f16, kind="ExternalOutput")

    # dt-scaled coefficients for the fused last-term matmuls (m1) and the
    # fused second-to-last-term matmuls (m2, targets z2..z5 only)
    cs = [float(np.float64(A_TAB[st + 1][st]) * dt) for st in range(5)]
    cs.append(float(np.float64(B_TAB[5]) * dt))
    cs2 = [float(np.float64(A_TAB[st + 2][st]) * dt) for st in range(4)]
    cs2.append(float(np.float64(B_TAB[4]) * dt))   # boundary group's m2

    with tile.TileContext(nc) as tc:
        with (
            tc.tile_pool(name="consts", bufs=1) as consts,
            tc.tile_pool(name="xs", bufs=3) as xs,
            tc.tile_pool(name="zs", bufs=8) as zs,
            tc.tile_pool(name="hs", bufs=2) as hs,
            tc.tile_pool(name="tmps", bufs=4) as tmps,
            tc.tile_pool(name="accs", bufs=12) as accs,
            tc.tile_pool(name="ph", bufs=3, space=bass.MemorySpace.PSUM) as ph,
            tc.tile_pool(name="pf", bufs=2, space=bass.MemorySpace.PSUM) as pf,
        ):
            # --- persistent data ---
            w1_t = consts.tile([KZ, H], bf16, tag="w1")
            w2k_t = consts.tile([HH, 2, D], bf16, tag="w2k")
            w2t_t = consts.tile([D, H], bf16, tag="w2t")
            uT_t = consts.tile([F, TU, BC], bf16, tag="uT")
            duT_t = consts.tile([F, TU, BC], bf16, tag="duT")
            x0_t = consts.tile([D, BC], f32, tag="x0")
            out_sb = consts.tile([D, NITER, BC], f16, tag="out")
            m_sb = consts.tile([HH, 2, 6, H], bf16, tag="m")
            nc.sync.dma_start(out=w1_t[:], in_=w1_d[:])
            nc.sync.dma_start(out=w2k_t[:], in_=w2k_d[:])
            nc.sync.dma_start(out=w2t_t[:], in_=w2t_d[:])
            # small first chunks unblock the first substeps' interps well
            # before the bulk of the forcing data lands
            nc.sync.dma_start(out=x0_t[:], in_=x0T_d[:])
            nc.sync.dma_start(out=uT_t[:, 0:16, :], in_=uT_d[:, 0:16, :])
            nc.sync.dma_start(out=duT_t[:, 0:16, :], in_=duT_d[:, 0:16, :])
            nc.sync.dma_start(out=uT_t[:, 16:TU, :], in_=uT_d[:, 16:TU, :])
            nc.sync.dma_start(out=duT_t[:, 16:TU, :], in_=duT_d[:, 16:TU, :])

            # --- build M = W2 @ W1x on device, then 6 c-scaled bf16 copies;
            # m_sb[:, k, ci, :] holds (cs[ci] * M)[k*128:(k+1)*128, :] ---
            mp = ph.tile([HH, 2, 512], f32, tag="hp")
            for k in range(2):
                nc.tensor.matmul(mp[:, k, 0:H], w2t_t[:, k * HH:(k + 1) * HH],
                                 w1_t[0:D, :], start=True, stop=True)
            for ci, c in enumerate(cs):
                for k in range(2):
                    nc.vector.tensor_scalar_mul(m_sb[:, k, ci, :],
                                                mp[:, k, 0:H], float(c))
            m2_sb = consts.tile([HH, 2, 5, H], bf16, tag="m2")
            for ci, c in enumerate(cs2):
                for k in range(2):
                    nc.vector.tensor_scalar_mul(m2_sb[:, k, ci, :],
                                                mp[:, k, 0:H], float(c))

            def put_u(z, gs):
                """Write forcing rows of z for global stage index gs via
                on-device linear interpolation. Two Pool ops (the Pool
                engine has no TensorScalarPtr in the V3 ISA); these are
                fully off the critical path."""
                i = int(idx[gs])
                tmp = tmps.tile([F, BC], bf16, tag="ut")
                nc.gpsimd.tensor_scalar_mul(tmp[:], duT_t[:, i, :],
                                            float(w[gs]))
                nc.gpsimd.tensor_tensor(out=z[D:KZ, :], in0=tmp[:],
                                        in1=uT_t[:, i, :], op=ADD)

            # --- first stage-0 z and its h_pre ---
            z0 = zs.tile([KZ, BC], bf16, tag="z")
            nc.gpsimd.tensor_copy(out=z0[0:D, :], in_=x0_t[:])
            put_u(z0, 0)
            hp = ph.tile([HH, 2, 512], f32, tag="hp")
            for half in range(2):
                nc.tensor.matmul(hp[:, half, 0:BC],
                                 w1_t[:, half * HH:(half + 1) * HH], z0[:],
                                 start=True, stop=True)

            def open_group(z_rhs, g=None, start=True):
                """Emit the W1 @ z matmuls for a future stage's h_pre group
                (allocating the PSUM tile unless the group was already
                started by m2 matmuls). Emitted at least a full stage
                before the closing c*M matmuls — so a late z stalls the
                in-order PE while it is idle anyway instead of blocking
                ready work behind it."""
                if g is None:
                    g = ph.tile([HH, 2, 512], f32, tag="hp")
                for half in range(2):
                    nc.tensor.matmul(g[:, half, 0:BC],
                                     w1_t[:, half * HH:(half + 1) * HH],
                                     z_rhs[:], start=start, stop=False)
                return g

            nsub_all = NITER * N_SUB
            x_cur = x0_t          # f32 x at current substep start (tile/AP)
            hp_n1 = None          # group for the next stage (openers emitted)

            for sub in range(nsub_all):
                it, j = divmod(sub, N_SUB)
                last = sub == nsub_all - 1
                boundary = j == N_SUB - 1      # writes out_sb at st5

                # z tiles for stages 3..5 of this substep (u rows now,
                # x rows filled by RK STTs below); stages 1/2 carry over
                # from the previous substep's st5 (x rows = bf16(x_new)
                # written there, skipping an extra copy hop)
                z_next = [None] * 6
                for st in range(3, 6):
                    z = zs.tile([KZ, BC], bf16, tag="z")
                    put_u(z, sub * 6 + st)
                    z_next[st] = z
                if sub == 0:
                    for st in (1, 2):
                        z = zs.tile([KZ, BC], bf16, tag="z")
                        put_u(z, st)
                        nc.gpsimd.tensor_copy(out=z[0:D, :],
                                              in_=x_cur[0:D, :])
                        z_next[st] = z
                else:
                    z_next[1], z_next[2] = z1_carry, z2_carry
                # next substep's stage-0/1/2 z tiles (x rows at st4/st5)
                if not last:
                    z1n = zs.tile([KZ, BC], bf16, tag="z")
                    put_u(z1n, (sub + 1) * 6)
                    z1c = zs.tile([KZ, BC], bf16, tag="z")
                    put_u(z1c, (sub + 1) * 6 + 1)
                    z2c = zs.tile([KZ, BC], bf16, tag="z")
                    put_u(z2c, (sub + 1) * 6 + 2)
                else:
                    z1n = z1c = z2c = None

                hp_n1 = open_group(z_next[1])  # stage-1 group openers

                acc = {tt: x_cur for tt in range(3, 6)}
                accx = x_cur
                x_new = xs.tile([D, BC], f32, tag="x")

                for st in range(6):
                    # ---- tanh (PSUM -> SBUF bf16) ----
                    h_sb = hs.tile([HH, 2, BC], bf16, tag="h")
                    nc.scalar.activation(h_sb[:], hp[:, :, 0:BC], FP.Tanh)

                    # ---- close the next group: c*M @ h (fused last term;
                    # its W1 openers were emitted a stage ago) ----
                    if hp_n1 is not None:
                        for o in range(2):
                            for k in range(2):
                                nc.tensor.matmul(
                                    hp_n1[:, o, 0:BC],
                                    m_sb[:, k, st, o * HH:(o + 1) * HH],
                                    h_sb[:, k, :], start=False, stop=(k == 1))

                    # ---- f_st = W2.T @ h -> PSUM ----
                    fp_t = pf.tile([D, BC], f32, tag="f")
                    for k in range(2):
                        nc.tensor.matmul(fp_t[:], w2k_t[:, k, :],
                                         h_sb[:, k, :], start=(k == 0),
                                         stop=(k == 1))

                    # ---- start the group two stages ahead: fused
                    # second-to-last term c2*M @ h, then its W1 openers
                    # (the z's x rows were finished a stage ago) ----
                    if st <= 3:
                        z_ahead = z_next[st + 2]
                    elif st == 4 and not last:
                        z_ahead = z1n            # next substep's stage 0
                    else:
                        z_ahead = None
                    if z_ahead is not None:
                        hp_n2 = ph.tile([HH, 2, 512], f32, tag="hp")
                        for o in range(2):
                            for k in range(2):
                                nc.tensor.matmul(
                                    hp_n2[:, o, 0:BC],
                                    m2_sb[:, k, st, o * HH:(o + 1) * HH],
                                    h_sb[:, k, :], start=(k == 0), stop=False)
                        open_group(z_ahead, g=hp_n2, start=False)
                    else:
                        hp_n2 = None

                    def axpy(out, c, in1):
                        nc.vector.scalar_tensor_tensor(
                            out=out, in0=fp_t[:], scalar=c, in1=in1,
                            op0=MULT, op1=ADD)

                    # ---- RK partial-sum updates touching f_st (the write
                    # completing z_{st+3} comes first; the last TWO terms
                    # of every z2..z5 arrive fused via m2/m1) ----
                    for tt in range(st + 3, 6):
                        a = A_TAB[tt][st]
                        if a == 0.0:
                            continue
                        c = float(np.float64(a) * dt)
                        if st == tt - 3:
                            # final partial term -> bf16 into stage-tt z
                            axpy(z_next[tt][0:D, :], c, acc[tt][0:D, :])
                        else:
                            nacc = accs.tile([D, BC], f32, tag="acc")
                            axpy(nacc[:], c, acc[tt][0:D, :])
                            acc[tt] = nacc
                    bcoef = B_TAB[st]
                    if bcoef != 0.0:
                        c = float(np.float64(bcoef) * dt)
                        if st == 3:
                            # x' minus its last two terms (b4/b5 fused via
                            # m2/m1): bf16 into next substep's z0
                            if not last:
                                axpy(z1n[0:D, :], c, accx[0:D, :])
                            nacc = accs.tile([D, BC], f32, tag="acc")
                            axpy(nacc[:], c, accx[0:D, :])
                            accx = nacc
                        elif st == 5:
                            if z1c is not None:
                                # next substep's stage-1 x rows: bf16(x_new)
                                # written first so its W1 openers fire early
                                axpy(z1c[0:D, :], c, accx[0:D, :])
                                nc.gpsimd.tensor_copy(out=z2c[0:D, :],
                                                      in_=z1c[0:D, :])
                            axpy(x_new, c, accx[0:D, :])
                        else:
                            nacc = accs.tile([D, BC], f32, tag="acc")
                            axpy(nacc[:], c, accx[0:D, :])
                            accx = nacc

                    # ---- rotate groups ----
                    hp = hp_n1
                    hp_n1 = hp_n2

                if boundary:
                    # f16 output snapshot; the f32 chain continues via x_new
                    nc.gpsimd.tensor_copy(out=out_sb[:, it, :], in_=x_new[:])
                x_cur = x_new
                z1_carry, z2_carry = z1c, z2c

            nc.sync.dma_start(out=out_d[:], in_=out_sb[:])

    nc.compile()
    return nc


def _prep_inputs(x0, t_eval, t_u, u_batch, W1, W2):
    # u/du in the 64-partition layout: row 8*(k%8)+f, col k//8 = u[:, k, f]
    uF = u_batch.transpose(2, 1, 0)                       # [F, TU, B]
    du = np.zeros_like(uF)
    du[:, :TU - 1, :] = uF[:, 1:, :] - uF[:, :-1, :]

    uT = np.ascontiguousarray(uF).astype(ml_dtypes.bfloat16)
    duT = np.ascontiguousarray(du).astype(ml_dtypes.bfloat16)
    w1 = W1.astype(ml_dtypes.bfloat16)                    # [72, 256]
    w2k = np.ascontiguousarray(
        W2.reshape(2, HH, D).transpose(1, 0, 2)).astype(
        ml_dtypes.bfloat16)                               # [128, 2, 64]
    w2t = np.ascontiguousarray(W2.T).astype(ml_dtypes.bfloat16)  # [64, 256]
    return uT, duT, w1, w2k, w2t


def kernel(x0, t_eval, t_u, u_batch, W1, b1, W2, b2):
    x0 = np.asarray(x0, np.float32)
    t_eval = np.asarray(t_eval, np.float32)
    t_u = np.asarray(t_u, np.float32)
    u_batch = np.asarray(u_batch, np.float32)
    W1 = np.asarray(W1, np.float32)
    b1 = np.asarray(b1, np.float32)
    W2 = np.asarray(W2, np.float32)
    b2 = np.asarray(b2, np.float32)
    assert not np.any(b1 != 0.0) and not np.any(b2 != 0.0), \
        "v3 kernel assumes zero biases (guaranteed by setup_inputs)"

    ts, dts = _host_times(t_eval)
    dt = float(np.float64(dts).mean())
    assert np.ptp(np.float64(dts)) <= 1e-4 * abs(dt) + 1e-12, \
        "non-uniform t_eval grid not supported"
    idx, w = _interp_consts(t_eval, t_u)

    key = (dt, t_eval.tobytes(), t_u.tobytes(), NITER)
    if key not in _CACHE:
        _CACHE[key] = _build_program(dt, idx, w)
    nc = _CACHE[key]

    uT, duT, w1, w2k, w2t = _prep_inputs(x0, t_eval, t_u, u_batch, W1, W2)

    in_maps = []
    for c in range(NCORES):
        bsl = slice(c * BC, (c + 1) * BC)
        in_maps.append({
            "x0T": np.ascontiguousarray(x0[bsl].T),
            "uT": np.ascontiguousarray(uT[:, :, bsl]),
            "duT": np.ascontiguousarray(duT[:, :, bsl]),
            "w1": w1, "w2k": w2k, "w2t": w2t,
        })

    trace = bool(int(os.environ.get("NODE_TRACE", "0")))
    old_m = nc.m
    nc.m = get_hw_module(nc.m)
    try:
        res = run_bass_kernel_spmd(nc, in_maps, list(range(NCORES)),
                                   trace=trace)
    finally:
        nc.m = old_m
    global LAST_RESULTS
    LAST_RESULTS = res

    out = np.empty((B, T, D), np.float32)
    out[:, 0, :] = x0
    for c in range(NCORES):
        bsl = slice(c * BC, (c + 1) * BC)
        # outT [D, NITER, BC] f16 -> [BC, NITER, D] f32
        out[bsl, 1:NITER + 1, :] = res.results[c]["outT"].astype(
            np.float32).transpose(2, 1, 0)
    return out


if __name__ == "__main__":
    import reference
    inputs = {k: np.asarray(v) for k, v in reference.setup_inputs().items()}
    got = kernel(**inputs)
    print("kernel output", got.shape, got.dtype)


# revision 55
# speedup vs baseline: 1.0218x; 1.0218x over previous
"""Trainium2 Bass kernel for a Neural ODE (dopri5, fixed substeps) — v3.

Problem: B=1024 trajectories of a D=64-dim ODE driven by an MLP
f(t,x) = tanh([x,u(t)] @ W1) @ W2, integrated with Dormand-Prince RK45
over 49 intervals x 4 substeps = 196 steps (6 MLP evals each).

Strategy (pure batch data-parallel, 8 cores x 128 batch), fully unrolled:
- Transposed layout: state xT [64,128] f32 (batch on free dim), stage
  inputs zT [72,128] bf16 (64 state rows + 8 forcing rows), hidden
  hT [128,2,128] bf16.
- Single-bf16 weights (no hi/lo split): tolerance is 2e-2, measured
  numerics land at ~1.4e-3.
- Forcing is interpolated ON DEVICE: raw u and its diffs (transposed
  [8,TU,128] bf16) are preloaded to SBUF in chunked DMAs (a small
  first chunk unblocks the first substeps early); each stage's u rows
  are two Pool ops with the interpolation weight baked as an immediate
  (t grids are known at build time; the program is cached on them).
- The last TWO RK terms of each stage's z are fused into its h_pre
  PSUM group via c*M matmuls (M = W2 @ W1x, built on device in the
  prologue): the m2 set (second-to-last term) starts each group two
  stages early, the m1 set closes it, keeping the serial chain at
  tanh -> 4 matmuls and giving every z x-row write (and the group's
  W1 openers) a full stage of slack against the in-order PE and the
  DVE queue. Stage periods sit at the 757 ns chain floor.
- Fully unrolled 196-step program: no For_i (no all-engine barriers, no
  per-iteration act-table reloads), no dynamic DMA descriptors. Outputs
  accumulate in SBUF (f16) and leave in ONE big DMA at the end; total
  DMA count is 9.
- RK linear combinations are DVE scalar_tensor_tensor AXPYs reading f
  straight from PSUM, with dt-scaled tableau coefficients baked as
  immediates.
"""

import os
import numpy as np
import ml_dtypes

import concourse.bass as bass
import concourse.bacc as bacc
import concourse.mybir as mybir
import concourse.tile as tile
from concourse.bass_utils import run_bass_kernel_spmd
from concourse.bass_interp import get_hw_module

NCORES = 8
B, D, F, H = 1024, 64, 8, 256
T, TU, N_SUB = 50, 128, 4
NITER = int(os.environ.get('NODE_NITER', T - 1))  # build fewer iters (dev)
BC = B // NCORES                   # 128 batch per core
KZ = D + F                         # 72 = state + forcing rows
HH = H // 2                        # 128

f32 = mybir.dt.float32
f16 = mybir.dt.float16
bf16 = mybir.dt.bfloat16
SUB = mybir.AluOpType.subtract
FP = mybir.ActivationFunctionType
MULT = mybir.AluOpType.mult
ADD = mybir.AluOpType.add

A_TAB = [
    [],
    [1 / 5],
    [3 / 40, 9 / 40],
    [44 / 45, -56 / 15, 32 / 9],
    [19372 / 6561, -25360 / 2187, 64448 / 6561, -212 / 729],
    [9017 / 3168, -355 / 33, 46732 / 5247, 49 / 176, -5103 / 18656],
]
B_TAB = [35 / 384, 0.0, 500 / 1113, 125 / 192, -2187 / 6784, 11 / 84]

_CACHE = {}
LAST_RESULTS = None


def _host_times(t_eval):
    """Substep times/dts exactly as the fp32 reference computes them."""
    t_eval = np.asarray(t_eval, np.float32)
    dtc = np.diff(t_eval)
    frac = (np.arange(N_SUB, dtype=np.float32) / np.float32(N_SUB))
    ts = (t_eval[:-1, None] + dtc[:, None] * frac).reshape(-1)
    dts = np.repeat(dtc / np.float32(N_SUB), N_SUB)
    return ts.astype(np.float32), dts.astype(np.float32)


def _stage_times(t, dt):
    """The 6 stage eval times for one step, mirroring the reference fp32."""
    t = np.float32(t)
    dt = np.float32(dt)
    return [
        t,
        t + dt / np.float32(5),
        t + np.float32(3) * dt / np.float32(10),
        t + np.float32(4) * dt / np.float32(5),
        t + np.float32(8) * dt / np.float32(9),
        t + dt,
    ]


def _interp_consts(t_eval, t_u):
    """(idx, w) fp32 interpolation constants for all NITER*N_SUB*6 stages."""
    ts, dts = _host_times(t_eval)
    nstage = NITER * N_SUB * 6
    tq = np.empty(nstage, np.float32)
    for s in range(NITER * N_SUB):
        tq[s * 6:(s + 1) * 6] = _stage_times(ts[s], dts[s])
    idx = np.clip(np.searchsorted(t_u, tq, side="right") - 1, 0, TU - 2)
    w = ((tq - t_u[idx]) / (t_u[idx + 1] - t_u[idx])).astype(np.float32)
    return idx.astype(np.int64), w


def _build_program(dt, idx, w):
    """Build the SPMD Bass program (identical on all cores), fully unrolled.

    dt: constant substep size baked into RK coefficients.
    idx/w: per-stage forcing interpolation constants (baked).
    """
    nc = bacc.Bacc("TRN2", target_bir_lowering=False, debug=False,
                   enable_asserts=False)

    x0T_d = nc.dram_tensor("x0T", [D, BC], f32, kind="ExternalInput")
    uT_d = nc.dram_tensor("uT", [F, TU, BC], bf16, kind="ExternalInput")
    duT_d = nc.dram_tensor("duT", [F, TU, BC], bf16, kind="ExternalInput")
    w1_d = nc.dram_tensor("w1", [KZ, H], bf16, kind="ExternalInput")
    w2k_d = nc.dram_tensor("w2k", [HH, 2, D], bf16, kind="ExternalInput")
    w2t_d = nc.dram_tensor("w2t", [D, H], bf16, kind="ExternalInput")
    # f16 output: halves device<->host traffic; the f32 state chain stays
    # on-device, outputs are rounded snapshots only
    out_d = nc.dram_tensor("outT", [D, NITER, BC], f16, kind="ExternalOutput")

    # dt-scaled coefficients for the fused last-term matmuls (m1) and the
    # fused second-to-last-term matmuls (m2, targets z2..z5 only)
    cs = [float(np.float64(A_TAB[st + 1][st]) * dt) for st in range(5)]
    cs.append(float(np.float64(B_TAB[5]) * dt))
    cs2 = [float(np.float64(A_TAB[st + 2][st]) * dt) for st in range(4)]
    cs2.append(float(np.float64(B_TAB[4]) * dt))   # boundary group's m2

    with tile.TileContext(nc) as tc:
        with (
            tc.tile_pool(name="consts", bufs=1) as consts,
            tc.tile_pool(name="xs", bufs=3) as xs,
            tc.tile_pool(name="zs", bufs=8) as zs,
            tc.tile_pool(name="hs", bufs=2) as hs,
            tc.tile_pool(name="tmps", bufs=4) as tmps,
            tc.tile_pool(name="accs", bufs=12) as accs,
            tc.tile_pool(name="ph", bufs=3, space=bass.MemorySpace.PSUM) as ph,
            tc.tile_pool(name="pf", bufs=2, space=bass.MemorySpace.PSUM) as pf,
        ):
            # --- persistent data ---
            w1_t = consts.tile([KZ, H], bf16, tag="w1")
            w2k_t = consts.tile([HH, 2, D], bf16, tag="w2k")
            w2t_t = consts.tile([D, H], bf16, tag="w2t")
            uT_t = consts.tile([F, TU, BC], bf16, tag="uT")
            duT_t = consts.tile([F, TU, BC], bf16, tag="duT")
            x0_t = consts.tile([D, BC], f32, tag="x0")
            out_sb = consts.tile([D, NITER, BC], f16, tag="out")
            m_sb = consts.tile([HH, 2, 6, H], bf16, tag="m")
            nc.sync.dma_start(out=w1_t[:], in_=w1_d[:])
            nc.sync.dma_start(out=w2k_t[:], in_=w2k_d[:])
            nc.sync.dma_start(out=w2t_t[:], in_=w2t_d[:])
            # small first chunks unblock the first substeps' interps well
            # before the bulk of the forcing data lands
            nc.sync.dma_start(out=x0_t[:], in_=x0T_d[:])
            nc.sync.dma_start(out=uT_t[:, 0:16, :], in_=uT_d[:, 0:16, :])
            nc.sync.dma_start(out=duT_t[:, 0:16, :], in_=duT_d[:, 0:16, :])
            nc.sync.dma_start(out=uT_t[:, 16:TU, :], in_=uT_d[:, 16:TU, :])
            nc.sync.dma_start(out=duT_t[:, 16:TU, :], in_=duT_d[:, 16:TU, :])

            # --- build M = W2 @ W1x on device, then 6 c-scaled bf16 copies;
            # m_sb[:, k, ci, :] holds (cs[ci] * M)[k*128:(k+1)*128, :] ---
            mp = ph.tile([HH, 2, 512], f32, tag="hp")
            for k in range(2):
                nc.tensor.matmul(mp[:, k, 0:H], w2t_t[:, k * HH:(k + 1) * HH],
                                 w1_t[0:D, :], start=True, stop=True)
            for ci, c in enumerate(cs):
                for k in range(2):
                    nc.vector.tensor_scalar_mul(m_sb[:, k, ci, :],
                                                mp[:, k, 0:H], float(c))
            m2_sb = consts.tile([HH, 2, 5, H], bf16, tag="m2")
            for ci, c in enumerate(cs2):
                for k in range(2):
                    nc.vector.tensor_scalar_mul(m2_sb[:, k, ci, :],
                                                mp[:, k, 0:H], float(c))

            def put_u(z, gs):
                """Write forcing rows of z for global stage index gs via
                on-device linear interpolation. Two Pool ops (the Pool
                engine has no TensorScalarPtr in the V3 ISA); these are
                fully off the critical path."""
                i = int(idx[gs])
                tmp = tmps.tile([F, BC], bf16, tag="ut")
                nc.gpsimd.tensor_scalar_mul(tmp[:], duT_t[:, i, :],
                                            float(w[gs]))
                nc.gpsimd.tensor_tensor(out=z[D:KZ, :], in0=tmp[:],
                                        in1=uT_t[:, i, :], op=ADD)

            # --- first stage-0 z and its h_pre ---
            z0 = zs.tile([KZ, BC], bf16, tag="z")
            nc.gpsimd.tensor_copy(out=z0[0:D, :], in_=x0_t[:])
            put_u(z0, 0)
            hp = ph.tile([HH, 2, 512], f32, tag="hp")
            for half in range(2):
                nc.tensor.matmul(hp[:, half, 0:BC],
                                 w1_t[:, half * HH:(half + 1) * HH], z0[:],
                                 start=True, stop=True)

            def open_group(z_rhs, g=None, start=True):
                """Emit the W1 @ z matmuls for a future stage's h_pre group
                (allocating the PSUM tile unless the group was already
                started by m2 matmuls). Emitted at least a full stage
                before the closing c*M matmuls — so a late z stalls the
                in-order PE while it is idle anyway instead of blocking
                ready work behind it."""
                if g is None:
                    g = ph.tile([HH, 2, 512], f32, tag="hp")
                for half in range(2):
                    nc.tensor.matmul(g[:, half, 0:BC],
                                     w1_t[:, half * HH:(half + 1) * HH],
                                     z_rhs[:], start=start, stop=False)
                return g

            nsub_all = NITER * N_SUB
            x_cur = x0_t          # f32 x at current substep start (tile/AP)
            hp_n1 = None          # group for the next stage (openers emitted)

            for sub in range(nsub_all):
                it, j = divmod(sub, N_SUB)
                last = sub == nsub_all - 1
                boundary = j == N_SUB - 1      # writes out_sb at st5

                # z tiles for stages 3..5 of this substep (u rows now,
                # x rows filled by RK STTs below); stages 1/2 carry over
                # from the previous substep's st5 (x rows = bf16(x_new)
                # written there, skipping an extra copy hop)
                z_next = [None] * 6
                for st in range(3, 6):
                    z = zs.tile([KZ, BC], bf16, tag="z")
                    put_u(z, sub * 6 + st)
                    z_next[st] = z
                if sub == 0:
                    for st in (1, 2):
                        z = zs.tile([KZ, BC], bf16, tag="z")
                        put_u(z, st)
                        nc.gpsimd.tensor_copy(out=z[0:D, :],
                                              in_=x_cur[0:D, :])
                        z_next[st] = z
                else:
                    z_next[1], z_next[2] = z1_carry, z2_carry
                # next substep's stage-0/1/2 z tiles (x rows at st4/st5)
                if not last:
                    z1n = zs.tile([KZ, BC], bf16, tag="z")
                    put_u(z1n, (sub + 1) * 6)
                    z1c = zs.tile([KZ, BC], bf16, tag="z")
                    put_u(z1c, (sub + 1) * 6 + 1)
                    z2c = zs.tile([KZ, BC], bf16, tag="z")
                    put_u(z2c, (sub + 1) * 6 + 2)
                else:
                    z1n = z1c = z2c = None

                hp_n1 = open_group(z_next[1])  # stage-1 group openers

                acc = {tt: x_cur for tt in range(3, 6)}
                accx = x_cur
                x_new = xs.tile([D, BC], f32, tag="x")

                for st in range(6):
                    # ---- tanh (PSUM -> SBUF bf16) ----
                    h_sb = hs.tile([HH, 2, BC], bf16, tag="h")
                    nc.scalar.activation(h_sb[:], hp[:, :, 0:BC], FP.Tanh)

                    # ---- close the next group: c*M @ h (fused last term;
                    # its W1 openers were emitted a stage ago) ----
                    if hp_n1 is not None:
                        for o in range(2):
                            for k in range(2):
                                nc.tensor.matmul(
                                    hp_n1[:, o, 0:BC],
                                    m_sb[:, k, st, o * HH:(o + 1) * HH],
                                    h_sb[:, k, :], start=False, stop=(k == 1))

                    # ---- f_st = W2.T @ h -> PSUM ----
                    fp_t = pf.tile([D, BC], f32, tag="f")
                    for k in range(2):
                        nc.tensor.matmul(fp_t[:], w2k_t[:, k, :],
                                         h_sb[:, k, :], start=(k == 0),
                                         stop=(k == 1))

                    # ---- start the group two stages ahead: fused
                    # second-to-last term c2*M @ h, then its W1 openers
                    # (the z's x rows were finished a stage ago) ----
                    if st <= 3:
                        z_ahead = z_next[st + 2]
                    elif st == 4 and not last:
                        z_ahead = z1n            # next substep's stage 0
                    else:
                        z_ahead = None
                    if z_ahead is not None:
                        hp_n2 = ph.tile([HH, 2, 512], f32, tag="hp")
                        for o in range(2):
                            for k in range(2):
                                nc.tensor.matmul(
                                    hp_n2[:, o, 0:BC],
                                    m2_sb[:, k, st, o * HH:(o + 1) * HH],
                                    h_sb[:, k, :], start=(k == 0), stop=False)
                        open_group(z_ahead, g=hp_n2, start=False)
                    else:
                        hp_n2 = None

                    def axpy(out, c, in1):
                        nc.vector.scalar_tensor_tensor(
                            out=out, in0=fp_t[:], scalar=c, in1=in1,
                            op0=MULT, op1=ADD)

                    # ---- RK partial-sum updates touching f_st (the write
                    # completing z_{st+3} comes first; the last TWO terms
                    # of every z2..z5 arrive fused via m2/m1) ----
                    for tt in range(st + 3, 6):
                        a = A_TAB[tt][st]
                        if a == 0.0:
                            continue
                        c = float(np.float64(a) * dt)
                        if st == tt - 3:
                            # final partial term -> bf16 into stage-tt z
                            axpy(z_next[tt][0:D, :], c, acc[tt][0:D, :])
                        else:
                            nacc = accs.tile([D, BC], f32, tag="acc")
                            axpy(nacc[:], c, acc[tt][0:D, :])
                            acc[tt] = nacc
                    bcoef = B_TAB[st]
                    if bcoef != 0.0:
                        c = float(np.float64(bcoef) * dt)
                        if st == 3:
                            # x' minus its last two terms (b4/b5 fused via
                            # m2/m1): bf16 into next substep's z0
                            if not last:
                                axpy(z1n[0:D, :], c, accx[0:D, :])
                            nacc = accs.tile([D, BC], f32, tag="acc")
                            axpy(nacc[:], c, accx[0:D, :])
                            accx = nacc
                        elif st == 5:
                            if z1c is not None:
                                # next substep's stage-1 x rows: bf16(x_new)
                                # written first so its W1 openers fire early
                                axpy(z1c[0:D, :], c, accx[0:D, :])
                                nc.gpsimd.tensor_copy(out=z2c[0:D, :],
                                                      in_=z1c[0:D, :])
                            axpy(x_new, c, accx[0:D, :])
                        else:
                            nacc = accs.tile([D, BC], f32, tag="acc")
                            axpy(nacc[:], c, accx[0:D, :])
                            accx = nacc

                    # ---- rotate groups ----
                    hp = hp_n1
                    hp_n1 = hp_n2

                if boundary:
                    # f16 output snapshot; the f32 chain continues via x_new
                    nc.gpsimd.tensor_copy(out=out_sb[:, it, :], in_=x_new[:])
                x_cur = x_new
                z1_carry, z2_carry = z1c, z2c

            nc.sync.dma_start(out=out_d[:], in_=out_sb[:])

    nc.compile()
    return nc


def _prep_inputs(x0, t_eval, t_u, u_batch, W1, W2):
    # u/du in the 64-partition layout: row 8*(k%8)+f, col k//8 = u[:, k, f]
    uF = u_batch.transpose(2, 1, 0)                       # [F, TU, B]
    du = np.zeros_like(uF)
    du[:, :TU - 1, :] = uF[:, 1:, :] - uF[:, :-1, :]

    uT = np.ascontiguousarray(uF).astype(ml_dtypes.bfloat16)
    duT = np.ascontiguousarray(du).astype(ml_dtypes.bfloat16)
    w1 = W1.astype(ml_dtypes.bfloat16)                    # [72, 256]
    w2k = np.ascontiguousarray(
        W2.reshape(2, HH, D).transpose(1, 0, 2)).astype(
        ml_dtypes.bfloat16)                               # [128, 2, 64]
    w2t = np.ascontiguousarray(W2.T).astype(ml_dtypes.bfloat16)  # [64, 256]
    return uT, duT, w1, w2k, w2t


def kernel(x0, t_eval, t_u, u_batch, W1, b1, W2, b2):
    x0 = np.asarray(x0, np.float32)
    t_eval = np.asarray(t_eval, np.float32)
    t_u = np.asarray(t_u, np.float32)
    u_batch = np.asarray(u_batch, np.float32)
    W1 = np.asarray(W1, np.float32)
    b1 = np.asarray(b1, np.float32)
    W2 = np.asarray(W2, np.float32)
    b2 = np.asarray(b2, np.float32)
    assert not np.any(b1 != 0.0) and not np.any(b2 != 0.0), \
        "v3 kernel assumes zero biases (guaranteed by setup_inputs)"

    ts, dts = _host_times(t_eval)
    dt = float(np.float64(dts).mean())
    assert np.ptp(np.float64(dts)) <= 1e-4 * abs(dt) + 1e-12, \
        "non-uniform t_eval grid not supported"
    idx, w = _interp_consts(t_eval, t_u)

    key = (dt, t_eval.tobytes(), t_u.tobytes(), NITER)
    if key not in _CACHE:
        _CACHE[key] = _build_program(dt, idx, w)
    nc = _CACHE[key]

    uT, duT, w1, w2k, w2t = _prep_inputs(x0, t_eval, t_u, u_batch, W1, W2)

    in_maps = []
    for c in range(NCORES):
        bsl = slice(c * BC, (c + 1) * BC)
        in_maps.append({
            "x0T": np.ascontiguousarray(x0[bsl].T),
            "uT": np.ascontiguousarray(uT[:, :, bsl]),
            "duT": np.ascontiguousarray(duT[:, :, bsl]),
            "w1": w1, "w2k": w2k, "w2t": w2t,
        })

    trace = bool(int(os.environ.get("NODE_TRACE", "0")))
    old_m = nc.m
    nc.m = get_hw_module(nc.m)
    try:
        res = run_bass_kernel_spmd(nc, in_maps, list(range(NCORES)),
                                   trace=trace)
    finally:
        nc.m = old_m
    global LAST_RESULTS
    LAST_RESULTS = res

    out = np.empty((B, T, D), np.float32)
    out[:, 0, :] = x0
    for c in range(NCORES):
        bsl = slice(c * BC, (c + 1) * BC)
        # outT [D, NITER, BC] f16 -> [BC, NITER, D] f32
        out[bsl, 1:NITER + 1, :] = res.results[c]["outT"].astype(
            np.float32).transpose(2, 1, 0)
    return out


if __name__ == "__main__":
    import reference
    inputs = {k: np.asarray(v) for k, v in reference.setup_inputs().items()}
    got = kernel(**inputs)
    print("kernel output", got.shape, got.dtype)
